# revision 9
# baseline (speedup 1.0000x reference)
"""Multi-head attention (B=2, S=2048, D=1024, H=16) on 8 Trainium2 cores.

Sharding: core = 4*b + g  (b = batch 0..1, g = head-group 0..3, 4 heads each).
Heads are processed in pairs; pair p covers the group's d-dims [128p, 128p+128).

Schedule: 64 pipelined rounds (one per (pair, qc, k-group)) keep the scalar
engine's exp stream and the tensor engine concurrently busy:

  round r: [normalize-b] [forced proj] scores(r) -> exp(r) -> PV(r-1)
           [normalize-a at sweep end] [V jit] [filler: proj / out-proj]

A short DMA-led preamble projects only K(pair0 sc0/sc1), Q(pair0,qc0),
V(kt0,1); all other projections and the output projection run as PE filler
in the rounds' slack so the tensor engine never idles (stays at full DVFS
pstate).  Sweep order is pair-major so pair1's projections have 4 sweeps of
slack to materialize.  Normalize is split into two phases one round apart
so its SBUF->SBUF z-gather DMA never head-blocks the vector-engine queue.
The last q-chunk's output projection is split by head-pair so only its
pair1 half (8 small matmuls + adds) remains after the final exp.

Exactness notes: b_k only shifts each softmax row uniformly -> dropped.
b_v and b_o commute with softmax-average -> folded into the host reduce.
b_q is applied on-device (fused into the Q PSUM->SBUF copy).
"""

import os
from collections import defaultdict, deque
from contextlib import ExitStack

import ml_dtypes
import numpy as np

import concourse.tile as tile
from concourse import bacc, mybir

B, S, D = 2, 2048, 1024
H, DH = 16, 64
NCORES = 8
NG = 4                  # head-group shards
DG = D // NG            # 256 dims per head-group (4 heads, 2 pairs)
P = 128
QC = 512                # q-chunk width
NQC = S // QC           # 4
NKT = S // P            # 16 k-tiles of 128
CD = D // P             # 8 contraction tiles for the projections
NR = 64                 # pipeline rounds: 2 pairs x 4 qc x 8 k-groups
F32 = mybir.dt.float32
BF16 = mybir.dt.bfloat16
AF = mybir.ActivationFunctionType
SCALE = 1.0 / float(np.sqrt(D))


def _body(ctx: ExitStack, tc: "tile.TileContext", io: dict):
    nc = tc.nc
    ctx.enter_context(nc.allow_low_precision(reason="bf16 matmul pipeline"))
    sb = ctx.enter_context(tc.tile_pool(name="sb", bufs=1))
    ps = ctx.enter_context(tc.tile_pool(name="ps", bufs=1, space="PSUM"))

    # ---------------- DMA: one ordered chain, earliest-needed first --------
    xk_sb, xq_sb, xv_sb = {}, {}, {}
    w_sb = {}

    def dma_x(dst_map, key, idx):
        t = sb.tile([P, CD, QC], BF16, tag="x", bufs=12, name=f"{key}{idx}")
        nc.sync.dma_start(t[:], io[key][idx])
        dst_map[idx] = t

    def dma_w(kind, pr):
        t = sb.tile([P, CD, P], BF16, tag="w", bufs=5, name=f"w{kind}{pr}")
        nc.sync.dma_start(t[:], io[f"w{kind}"][pr])
        w_sb[(kind, pr)] = t

    dma_w("k", 0)
    dma_x(xk_sb, "xk", 0)
    dma_w("q", 0)
    bq = sb.tile([P, 2], F32, tag="bq", bufs=1, name="bq")
    nc.sync.dma_start(bq[:], io["bq"])
    ones2 = sb.tile([P, 2], BF16, tag="ones2", bufs=1, name="ones2")
    nc.sync.dma_start(ones2[:], io["ones2"])
    dma_x(xq_sb, "xq", 0)
    dma_x(xk_sb, "xk", 1)
    wv = sb.tile([P, CD, DG], BF16, tag="wv", bufs=1, name="wv")
    nc.sync.dma_start(wv[:], io["wv"])
    dma_x(xv_sb, "xv", 0)
    dma_x(xv_sb, "xv", 1)
    dma_x(xk_sb, "xk", 2)
    dma_x(xv_sb, "xv", 2)
    dma_x(xk_sb, "xk", 3)
    dma_x(xv_sb, "xv", 3)
    dma_x(xq_sb, "xq", 1)
    dma_x(xq_sb, "xq", 2)
    dma_w("k", 1)
    dma_w("q", 1)
    dma_x(xq_sb, "xq", 3)
    woT = []
    for pr in range(2):
        t = sb.tile([P, D], BF16, tag="wo", bufs=2, name=f"woT{pr}")
        nc.sync.dma_start(t[:], io["wo"][pr])
        woT.append(t)

    # ---------------- projection / out-proj emitters -----------------------
    KT, QT, V, UN, YSB, YP32 = {}, {}, {}, {}, {}, {}

    def emit_qk_group(kind, pr, idx):
        w = w_sb[(kind, pr)]
        x = (xk_sb if kind == "k" else xq_sb)[idx]
        pg = ps.tile([P, QC], F32, tag="pj", bufs=1, name=f"pg{kind}{pr}{idx}")
        for c in range(CD):
            nc.tensor.matmul(
                pg[:], w[:, c, :], x[:, c, :], start=(c == 0), stop=(c == CD - 1)
            )
        t = sb.tile([P, QC], BF16, tag=f"{kind}t", bufs=8, name=f"{kind}T{pr}_{idx}")
        if kind == "q":
            nc.vector.tensor_scalar_add(t[:], pg[:], bq[:, pr : pr + 1])
            QT[(pr, idx)] = t
        else:
            nc.vector.tensor_copy(t[:], pg[:])
            KT[(pr, idx)] = t

    def emit_v_group(kt):
        # both head-pairs at once: [128 k, 256 d] PSUM, two V_aug tiles out
        sc, off = divmod(kt, 4)
        x = xv_sb[sc]
        pg = ps.tile([P, DG], F32, tag="pj", bufs=1, name=f"pgv{kt}")
        for c in range(CD):
            nc.tensor.matmul(
                pg[:],
                x[:, c, off * P : (off + 1) * P],
                wv[:, c, :],
                start=(c == 0),
                stop=(c == CD - 1),
            )
        for pair in (0, 1):
            vt = sb.tile([P, 2, DH + 1], BF16, tag="v", bufs=32, name=f"V{pair}_{kt}")
            nc.vector.tensor_copy(
                vt[:, :, 0:DH],
                pg[:, pair * P : (pair + 1) * P].rearrange("p (i d) -> p i d", i=2),
            )
            nc.vector.tensor_copy(vt[:, :, DH : DH + 1], ones2[:, :, None])
            V[(pair, kt)] = vt

    pending = deque()

    def emit_outproj_unit(tag="pj"):
        qc, qi, ec = pending.popleft()
        qt = qc * 4 + qi
        if ec == 0:
            YSB[qt] = sb.tile([P, D], BF16, tag="y", bufs=4, name=f"Y{qt}")
        ysb = YSB[qt]
        yp = ps.tile(
            [P, QC], F32, tag=tag, bufs=(1 if tag == "pj" else 3), name=f"yp{qt}_{ec}"
        )
        for pr in range(2):
            nc.tensor.matmul(
                yp[:],
                UN[(qc, pr)][:, qi * P : (qi + 1) * P],
                woT[pr][:, ec * QC : (ec + 1) * QC],
                start=(pr == 0),
                stop=(pr == 1),
            )
        nc.vector.tensor_copy(ysb[:, ec * QC : (ec + 1) * QC], yp[:])
        if ec == 1:
            nc.sync.dma_start(io["y"][qt * P : (qt + 1) * P, :], ysb[:])

    half_pending = deque()

    def emit_outproj_half(tag="pj"):
        # pair0 half of a qc3 unit: partial kept in f32 SBUF
        qi, ec = half_pending.popleft()
        qt = 12 + qi
        if ec == 0:
            YP32[qt] = sb.tile([P, D], BF16, tag="y3", bufs=4, name=f"YP{qt}")
        yp = ps.tile(
            [P, QC], F32, tag=tag, bufs=(1 if tag == "pj" else 3), name=f"yh{qt}_{ec}"
        )
        nc.tensor.matmul(
            yp[:],
            UN[(3, 0)][:, qi * P : (qi + 1) * P],
            woT[0][:, ec * QC : (ec + 1) * QC],
            start=True,
            stop=True,
        )
        nc.vector.tensor_copy(YP32[qt][:, ec * QC : (ec + 1) * QC], yp[:])
        YSB[qt] = YP32[qt]

    def emit_outproj_complete(qi, ec, tag):
        qt = 12 + qi
        ysb = YP32[qt]
        yp = ps.tile(
            [P, QC], F32, tag=tag, bufs=(1 if tag == "pj" else 3), name=f"yc{qt}_{ec}"
        )
        nc.tensor.matmul(
            yp[:],
            UN[(3, 1)][:, qi * P : (qi + 1) * P],
            woT[1][:, ec * QC : (ec + 1) * QC],
            start=True,
            stop=True,
        )
        nc.vector.tensor_add(
            ysb[:, ec * QC : (ec + 1) * QC],
            yp[:],
            ysb[:, ec * QC : (ec + 1) * QC],
        )
        if ec == 1:
            nc.sync.dma_start(io["y"][qt * P : (qt + 1) * P, :], ysb[:])

    # ---------------- attention round emitters -----------------------------
    PTs, U = {}, {}
    ZT = {}

    def sweep_of(r):
        s = r // 8
        pair, qc = divmod(s, 4)
        return s, pair, qc, r % 8

    def emit_scores(r):
        _, pair, qc, kg = sweep_of(r)
        for i in (0, 1):
            lo = 64 * i
            st = ps.tile([P, 2, QC], F32, tag="st", bufs=2, name=f"st{r}_{i}")
            for kk in (0, 1):
                kt = kg * 2 + kk
                sc, off = divmod(kt, 4)
                nc.tensor.matmul(
                    st[:, kk, :],
                    KT[(pair, sc)][lo : lo + 64, off * P : (off + 1) * P],
                    QT[(pair, qc)][lo : lo + 64, :],
                    start=True,
                    stop=True,
                    tile_position=(lo, 0),
                )
            pt = sb.tile([P, 2, QC], BF16, tag="pt", bufs=4, name=f"pt{r}_{i}")
            nc.scalar.activation(pt[:], st[:], AF.Exp, scale=SCALE)
            PTs[(r, i)] = pt

    def emit_pv(r):
        s, pair, qc, kg = sweep_of(r)
        if kg == 0:
            U[s] = [
                ps.tile([P, QC], F32, tag="u", bufs=3, name=f"U{s}_{i}")
                for i in (0, 1)
            ]
        for i in (0, 1):
            pt = PTs.pop((r, i))
            for kk in (0, 1):
                kt = kg * 2 + kk
                nc.tensor.matmul(
                    U[s][i][0:65, :],
                    V[(pair, kt)][:, i, :],
                    pt[:, kk, :],
                    start=(kg == 0 and kk == 0),
                    stop=(kg == 7 and kk == 1),
                )

    def emit_normalize_a(s):
        # denominator gather: DVE copy off PSUM row 64, then partition-shift
        for i in (0, 1):
            zr = sb.tile([65, QC], F32, tag="zr", bufs=2, name=f"zr{s}_{i}")
            nc.vector.tensor_copy(zr[64:65, :], U[s][i][64:65, :])
            z = sb.tile([1, QC], F32, tag="z", bufs=2, name=f"z{s}_{i}")
            nc.sync.dma_start(z[:], zr[64:65, :])
            ZT[(s, i)] = z

    def emit_normalize_b(s):
        pair, qc = divmod(s, 4)
        un = sb.tile([P, QC], BF16, tag="un", bufs=8, name=f"UN{qc}_{pair}")
        for i in (0, 1):
            z = ZT.pop((s, i))
            rz = sb.tile([1, QC], F32, tag="rz", bufs=2, name=f"rz{s}_{i}")
            nc.vector.reciprocal(rz[:], z[:])
            rb = sb.tile([64, QC], F32, tag="rb", bufs=2, name=f"rb{s}_{i}")
            nc.gpsimd.partition_broadcast(rb[:], rz[:], channels=64)
            if i == 0:
                nc.vector.tensor_mul(un[0:64, :], U[s][i][0:64, :], rb[:])
            else:
                tmp = sb.tile([64, QC], BF16, tag="untmp", bufs=2, name=f"ut{s}")
                nc.vector.tensor_mul(tmp[:], U[s][i][0:64, :], rb[:])
                nc.sync.dma_start(un[64:128, :], tmp[:])
        del U[s]
        UN[(qc, pair)] = un
        if pair == 1 and qc < 3:
            pending.extend((qc, qi, ec) for qi in range(4) for ec in range(2))
        if pair == 0 and qc == 3:
            half_pending.extend((qi, ec) for qi in range(4) for ec in range(2))

    # ---------------- static schedule --------------------------------------
    class Job:
        __slots__ = ("cols", "fn", "done")

        def __init__(self, cols, fn):
            self.cols, self.fn, self.done = cols, fn, False

        def run(self):
            if not self.done:
                self.done = True
                self.fn()

    def qk_job(kind, pr, idx):
        return Job(4096, lambda: emit_qk_group(kind, pr, idx))

    jobs = {}
    for kind, pr, idx in [
        ("q", 0, 1), ("q", 0, 2), ("q", 0, 3),
        ("k", 0, 2), ("k", 0, 3),
        ("k", 1, 0), ("k", 1, 1), ("k", 1, 2), ("k", 1, 3),
        ("q", 1, 0), ("q", 1, 1), ("q", 1, 2), ("q", 1, 3),
    ]:
        jobs[(kind, pr, idx)] = qk_job(kind, pr, idx)

    half_specs = [(qi, ec) for qi in range(4) for ec in range(2)]
    half_jobs = [Job(512, emit_outproj_half) for _ in range(8)]

    # EDF-ordered general filler queue with earliest-emission gates.
    fq = deque(
        [
            (8, jobs[("q", 0, 1)]),
            (9, jobs[("q", 1, 0)]),
            (10, jobs[("q", 0, 2)]),
            (10, jobs[("k", 1, 0)]),
            (11, jobs[("k", 1, 1)]),
            (12, jobs[("k", 1, 2)]),
            (12, jobs[("q", 0, 3)]),
            (13, jobs[("k", 1, 3)]),
            (14, jobs[("q", 1, 1)]),
            (15, jobs[("q", 1, 2)]),
            (16, jobs[("q", 1, 3)]),
        ]
        + [(40 + 2 * j, hj) for j, hj in enumerate(half_jobs)]
    )

    # mand_pre: tiles this round's scores read -> force before scores.
    mand_pre = defaultdict(list)
    mand_pre[2].append(jobs[("k", 0, 2)])
    mand_pre[4].append(jobs[("k", 0, 3)])
    mand_pre[8].append(jobs[("q", 0, 1)])
    mand_pre[16].append(jobs[("q", 0, 2)])
    mand_pre[24].append(jobs[("q", 0, 3)])
    mand_pre[32].append(jobs[("k", 1, 0)])
    mand_pre[32].append(jobs[("q", 1, 0)])
    mand_pre[34].append(jobs[("k", 1, 1)])
    mand_pre[36].append(jobs[("k", 1, 2)])
    mand_pre[38].append(jobs[("k", 1, 3)])
    mand_pre[40].append(jobs[("q", 1, 1)])
    mand_pre[48].append(jobs[("q", 1, 2)])
    mand_pre[56].append(jobs[("q", 1, 3)])

    # mand_post: V just-in-time (kt 2r+2, 2r+3 emitted in round r).
    mand_post = defaultdict(list)
    for r in range(7):
        mand_post[r].extend(
            Job(2048, (lambda k: (lambda: emit_v_group(k)))(kt))
            for kt in (2 * r + 2, 2 * r + 3)
        )

    # ---------------- preamble ---------------------------------------------
    emit_qk_group("k", 0, 0)
    emit_qk_group("k", 0, 1)
    emit_qk_group("q", 0, 0)
    emit_v_group(0)
    emit_v_group(1)

    # ---------------- main pipeline ----------------------------------------
    BUDGET = 1600
    for r in range(NR):
        if r % 8 == 1 and r > 8:
            emit_normalize_b(r // 8 - 1)
        for job in mand_pre[r]:
            job.run()
        emit_scores(r)
        if r > 0:
            emit_pv(r - 1)
        if r % 8 == 0 and r > 0:
            emit_normalize_a(r // 8 - 1)
        for job in mand_post[r]:
            job.run()
        budget = BUDGET
        while budget > 0:
            while fq and fq[0][1].done:
                fq.popleft()
            if fq and fq[0][0] <= r:
                _, job = fq.popleft()
                budget -= job.cols
                job.run()
            elif pending:
                emit_outproj_unit()
                budget -= 1024
            else:
                break

    # ---------------- drain -------------------------------------------------
    emit_pv(NR - 1)
    emit_normalize_a(7)
    for _, job in fq:
        job.run()
    while pending:
        emit_outproj_unit()
    while half_pending:
        emit_outproj_half()
    emit_normalize_b(7)
    tags = ["pj", "u", "u", "u"]
    for n, (qi, ec) in enumerate(half_specs):
        emit_outproj_complete(qi, ec, tags[n % 4])


def build_program():
    nc = bacc.Bacc(
        "TRN2", target_bir_lowering=False, debug=False, num_devices=NCORES
    )
    io = {
        "xq": nc.dram_tensor("xq", [NQC, P, CD, QC], BF16, kind="ExternalInput").ap(),
        "xk": nc.dram_tensor("xk", [NQC, P, CD, QC], BF16, kind="ExternalInput").ap(),
        "xv": nc.dram_tensor("xv", [NQC, P, CD, QC], BF16, kind="ExternalInput").ap(),
        "wq": nc.dram_tensor("wq", [2, P, CD, P], BF16, kind="ExternalInput").ap(),
        "wk": nc.dram_tensor("wk", [2, P, CD, P], BF16, kind="ExternalInput").ap(),
        "wv": nc.dram_tensor("wv", [P, CD, DG], BF16, kind="ExternalInput").ap(),
        "wo": nc.dram_tensor("wo", [2, P, D], BF16, kind="ExternalInput").ap(),
        "bq": nc.dram_tensor("bq", [P, 2], F32, kind="ExternalInput").ap(),
        "ones2": nc.dram_tensor("ones2", [P, 2], BF16, kind="ExternalInput").ap(),
        "y": nc.dram_tensor("y", [S, D], BF16, kind="ExternalOutput").ap(),
    }
    with tile.TileContext(nc) as tc:
        with ExitStack() as ctx:
            _body(ctx, tc, io)
    nc.compile()
    return nc


_CACHE = {}


def _get_program():
    if "nc" not in _CACHE:
        _CACHE["nc"] = build_program()
    return _CACHE["nc"]


def make_in_maps(inputs):
    q = np.asarray(inputs["query"], np.float32)
    k = np.asarray(inputs["key"], np.float32)
    v = np.asarray(inputs["value"], np.float32)
    W_q = np.asarray(inputs["W_q"], np.float32)
    W_k = np.asarray(inputs["W_k"], np.float32)
    W_v = np.asarray(inputs["W_v"], np.float32)
    W_o = np.asarray(inputs["W_o"], np.float32)
    b_q = np.asarray(inputs["b_q"], np.float32)

    bf = ml_dtypes.bfloat16

    def xblocks(x):  # [S, D] activations -> [blk, p, c, s] with x.T blocked
        xt = np.ascontiguousarray(x.T).astype(bf)  # [D, S]
        return np.ascontiguousarray(
            xt.reshape(CD, P, NQC, QC).transpose(2, 1, 0, 3)
        )

    def wblocks(w_sl):  # [D, 256] (= W[sl].T) -> [pr, p, c, d]
        return np.ascontiguousarray(
            w_sl.reshape(CD, P, 2, P).transpose(2, 1, 0, 3).astype(bf)
        )

    xq = [xblocks(q[b]) for b in range(B)]
    xk = [xblocks(k[b]) for b in range(B)]
    xv = [xblocks(v[b]) for b in range(B)]

    in_maps = []
    for core in range(NCORES):
        b, g = divmod(core, NG)
        sl = slice(g * DG, (g + 1) * DG)
        in_maps.append(
            {
                "xq": xq[b],
                "xk": xk[b],
                "xv": xv[b],
                "wq": wblocks(W_q[sl, :].T),
                "wk": wblocks(W_k[sl, :].T),
                "wv": np.ascontiguousarray(
                    W_v[sl, :].T.reshape(CD, P, DG).transpose(1, 0, 2).astype(bf)
                ),
                "wo": np.ascontiguousarray(
                    W_o[:, sl].T.reshape(2, P, D).astype(bf)
                ),
                "bq": np.ascontiguousarray(b_q[sl].reshape(2, P).T),
                "ones2": np.ones((P, 2), bf),
            }
        )
    return in_maps


def kernel(**inputs):
    from concourse.bass_utils import run_bass_kernel_spmd

    nc = _get_program()
    in_maps = make_in_maps(inputs)
    trace = bool(int(os.environ.get("MHA_TRACE", "0")))
    res = run_bass_kernel_spmd(nc, in_maps, list(range(NCORES)), trace=trace)
    _CACHE["last_results"] = res

    W_o = np.asarray(inputs["W_o"], np.float64)
    b_o = np.asarray(inputs["b_o"], np.float64)
    b_v = np.asarray(inputs["b_v"], np.float64)
    out = np.zeros((B, S, D), np.float32)
    for core in range(NCORES):
        b = core // NG
        out[b] += res.results[core]["y"].astype(np.float32)
    # b_v and b_o commute with the attention average / output projection.
    out += (b_o + b_v @ W_o.T).astype(np.float32)[None, None, :]
    return out
